# revision 1
# baseline (speedup 1.0000x reference)
"""NNUE evaluation kernel for Trainium2 (8 NeuronCores, data-parallel batch).

reference math:
    wh = clip(white @ W_ft.T, 0, 1)        # [B, 256]
    bh = clip(black @ W_ft.T, 0, 1)        # [B, 256]
    x  = concat(wh, bh)                    # [B, 512]
    x  = relu(x @ W1.T + b1); x = relu(x @ W2.T + b2)
    ev = (x @ W3.T + b3) * stm[:, None]    # [B, 1]

Strategy: shard B=4096 across 8 cores (512 rows each), data-parallel, no
collectives. Per core the two [512, 40960] feature GEMMs read ~210 MB of
fp32 from HBM (features + W_ft) -> ~590 us roofline at ~358 GB/s; the
kernel measures ~640-670 us. The contraction dim (40960) is contiguous in
DRAM for both operands, so feature tiles are transposed on-chip via PE
matmul-with-identity into PSUM and evacuated to SBUF by DVE; W_ft.T and
the MLP weights are pre-transposed (and descriptor-friendly pre-swizzled)
on the host. The GEMM then runs with W_ft.T tiles stationary and feat.T
[k, b=512] moving in float32r (single-pass fp32_mode=HIGH, 1 cycle/row,
~tf32 precision; everything feeding an f32r matmul must itself be
declared f32r for the BIR verifier), accumulating out.T [h, b] in PSUM
across all 320 k-tiles. The clip is fused into the PSUM evacuation and
the tiny MLP stays in transposed [features, batch] layout to the end.

This walrus build rejects instructions with >1 sync wait, so a post-pass
(_split_multi_waits) redistributes Tile-emitted waits onto single-wait
no-ops.
"""

import sys
import types

import numpy as np


def _inject_ntff_hook():
    """Register the axon NTFF profile hook if this image's antenv lacks it."""
    try:
        import antenv.axon_hooks  # noqa: F401
        return
    except ImportError:
        pass
    try:
        import trn_agent_boot.trn_boot as tb
        hook = tb._ntff_profile_via_ctypes("/opt/axon/libaxon_pjrt.so")
    except Exception:
        hook = None
    mod = types.ModuleType("antenv.axon_hooks")
    mod.get_axon_ntff_profile_hook = lambda: hook
    mod.set_axon_ntff_profile_hook = lambda h: None
    sys.modules["antenv.axon_hooks"] = mod


_inject_ntff_hook()

import concourse.bass as bass
import concourse.mybir as mybir
from concourse.masks import make_identity
from concourse.tile import TileContext

N_CORES = 8
B = 4096
BS = B // N_CORES          # 512 batch rows per core
IN = 40960                 # feature count (contraction dim)
H = 256                    # hidden per perspective
KC = 2048                  # k-slab width
NSLAB = IN // KC           # 20
KT = KC // 128             # k-tiles per slab: 16

F32 = mybir.dt.float32
F32R = mybir.dt.float32r

def _split_multi_waits(nc: bass.Bass) -> None:
    """This walrus build rejects instructions carrying more than one sync
    wait. Split any such instruction: emit single-wait no-ops on the same
    engine immediately before it (same engine stream => same semantics)."""
    for f in nc.m.functions:
        for bb in f.blocks:
            new_insts = []
            changed = False
            for inst in bb.instructions:
                si = inst.sync_info
                waits = list(si.on_wait) if si is not None and si.on_wait else []
                if len(waits) > 1:
                    changed = True
                    for i, w in enumerate(waits[:-1]):
                        nop = mybir.InstNoOp(
                            name=f"{inst.name}-sw{i}", ins=[], outs=[]
                        )
                        nop.engine = inst.engine
                        nop.sync_info = mybir.SyncInfo(on_wait=[w], on_update=[])
                        nc.register_instruction(nop)
                        new_insts.append(nop)
                    inst.sync_info = mybir.SyncInfo(
                        on_wait=[waits[-1]],
                        on_update=list(si.on_update) if si.on_update else [],
                    )
                new_insts.append(inst)
            if changed:
                bb.instructions = new_insts


def build_kernel(mm_f32r: bool = True, tr_f32r: bool = True) -> bass.Bass:
    # CD ("compute dtype") is the dtype of everything on the feature-GEMM
    # path: float32r is fp32 storage that the PE reads at full rate (vs 1/4
    # for plain fp32); the BIR verifier requires every producer on the path
    # to also be declared float32r ("rounded").
    CD = F32R if mm_f32r else F32

    def mm_cast(ap):
        return ap

    def tr_cast(ap):
        return ap

    nc = bass.Bass()

    # weights arrive pre-transposed from the host (part of the sharding
    # prep): W_ftT = W_ft.T [IN, H], W1T = W1.T [512, 32], etc.
    wf = nc.dram_tensor("white_features", [BS, IN], CD, kind="ExternalInput")
    bf = nc.dram_tensor("black_features", [BS, IN], CD, kind="ExternalInput")
    # W_ftTs: W_ft.T swizzled host-side to [NSLAB*128, KT*256] so each
    # slab's DMA reads 16 KB contiguous per partition (128 descriptors).
    w_ftTs = nc.dram_tensor(
        "W_ftTs", [NSLAB * 128, KT * H], CD, kind="ExternalInput")
    w1Ts = nc.dram_tensor("W1Ts", [128, 128], CD, kind="ExternalInput")
    b1 = nc.dram_tensor("b1", [32, 1], F32, kind="ExternalInput")
    w2T = nc.dram_tensor("W2T", [32, 32], CD, kind="ExternalInput")
    b2 = nc.dram_tensor("b2", [32, 1], F32, kind="ExternalInput")
    w3T = nc.dram_tensor("W3T", [32, 1], CD, kind="ExternalInput")
    b3 = nc.dram_tensor("b3", [1, 1], F32, kind="ExternalInput")
    stm = nc.dram_tensor("side_to_move", [1, BS], F32, kind="ExternalInput")
    out = nc.dram_tensor("evaluation", [1, BS], F32, kind="ExternalOutput")

    feats = [wf, bf]

    with TileContext(nc) as tc:
        with (
            tc.tile_pool(name="consts", bufs=1) as consts,
            tc.tile_pool(name="ot_psum", bufs=1, space="PSUM") as ot_pool,
            tc.tile_pool(name="mlp", bufs=1) as mlp,
        ):
            identity_f32 = consts.tile([128, 128], F32)
            make_identity(nc, identity_f32)
            identity = consts.tile([128, 128], CD)
            nc.vector.tensor_copy(out=identity[:], in_=identity_f32[:])

            # out.T accumulators: [h-tile 128, b 512] x (2 sides x 2 h-tiles)
            ot = [
                ot_pool.tile([128, BS], F32, tag=f"ot{i}", name=f"ot{i}")
                for i in range(4)
            ]

            # ---- main loop: feature-transformer GEMMs ----
            with (
                tc.tile_pool(name="fslab", bufs=4) as fslab_pool,
                tc.tile_pool(name="wt", bufs=2) as wt_pool,
                tc.tile_pool(name="ft", bufs=6) as ft_pool,
                tc.tile_pool(name="tr_psum", bufs=3, space="PSUM") as trp,
            ):
                for s in range(NSLAB):
                    k0 = s * KC
                    # W_ft.T slab (host-swizzled): contiguous [128, KT*256]
                    wt = wt_pool.tile([128, KT, 2 * 128], CD, tag="wt")
                    nc.scalar.dma_start(
                        out=wt[:],
                        in_=w_ftTs[s * 128:(s + 1) * 128, :],
                    )

                    for side in range(2):
                        f_nat = [
                            fslab_pool.tile(
                                [128, KC], CD, tag=f"fslab{bt}",
                                name=f"fnat{bt}",
                            )
                            for bt in range(4)
                        ]
                        for bt in range(4):
                            nc.sync.dma_start(
                                out=f_nat[bt][:],
                                in_=feats[side][
                                    bt * 128:(bt + 1) * 128, k0:k0 + KC
                                ],
                            )
                        for kt in range(KT):
                            pf = trp.tile([128, BS], CD, tag="ftr")
                            for bt in range(4):
                                nc.tensor.transpose(
                                    tr_cast(pf[:, bt * 128:(bt + 1) * 128]),
                                    tr_cast(
                                        f_nat[bt][:, kt * 128:(kt + 1) * 128]
                                    ),
                                    tr_cast(identity),
                                )
                            ft = ft_pool.tile([128, BS], CD, tag="ft")
                            nc.vector.tensor_copy(out=ft[:], in_=pf[:])
                            first = s == 0 and kt == 0
                            last = s == NSLAB - 1 and kt == KT - 1
                            for h in range(2):
                                nc.tensor.matmul(
                                    ot[side * 2 + h],
                                    mm_cast(
                                        wt[:, kt, h * 128:(h + 1) * 128]
                                    ),
                                    mm_cast(ft[:]),
                                    start=first,
                                    stop=last,
                                )

            # ---- MLP weight prep (emitted late so these DMAs schedule
            # behind the feature stream, not ahead of it) ----
            w1t = mlp.tile([128, 4, 32], CD)
            nc.scalar.dma_start(out=w1t[:], in_=w1Ts[:, :])
            w2t = mlp.tile([32, 32], CD)
            nc.scalar.dma_start(out=w2t[:], in_=w2T[:, :])
            w3t = mlp.tile([32, 1], CD)
            nc.scalar.dma_start(out=w3t[:], in_=w3T[:, :])
            b1_sb = mlp.tile([32, 1], F32)
            nc.scalar.dma_start(out=b1_sb[:], in_=b1[:, :])
            b2_sb = mlp.tile([32, 1], F32)
            nc.scalar.dma_start(out=b2_sb[:], in_=b2[:, :])
            b3_sb = mlp.tile([1, 1], F32)
            nc.scalar.dma_start(out=b3_sb[:], in_=b3[:, :])
            stm_sb = mlp.tile([1, BS], F32)
            nc.scalar.dma_start(out=stm_sb[:], in_=stm[:, :])

            # ---- clip + MLP (transposed layout throughout) ----
            with tc.tile_pool(name="mlp2_psum", bufs=1, space="PSUM") as mpp2:
                xt = []
                for i in range(4):
                    t = mlp.tile([128, BS], CD, tag=f"xt{i}")
                    nc.vector.tensor_scalar(
                        out=t[:], in0=ot[i][:], scalar1=0.0, scalar2=1.0,
                        op0=mybir.AluOpType.max, op1=mybir.AluOpType.min,
                    )
                    xt.append(t)

                h1p = mpp2.tile([32, BS], F32, tag="h1")
                for kt in range(4):
                    nc.tensor.matmul(
                        h1p, mm_cast(w1t[:, kt, :]), mm_cast(xt[kt][:]),
                        start=kt == 0, stop=kt == 3,
                    )
                h1 = mlp.tile([32, BS], CD)
                nc.vector.tensor_scalar(
                    out=h1[:], in0=h1p[:], scalar1=b1_sb[:, :], scalar2=0.0,
                    op0=mybir.AluOpType.add, op1=mybir.AluOpType.max,
                )

                h2p = mpp2.tile([32, BS], F32, tag="h2")
                nc.tensor.matmul(
                    h2p, mm_cast(w2t[:]), mm_cast(h1[:]), start=True, stop=True
                )
                h2 = mlp.tile([32, BS], CD)
                nc.vector.tensor_scalar(
                    out=h2[:], in0=h2p[:], scalar1=b2_sb[:, :], scalar2=0.0,
                    op0=mybir.AluOpType.add, op1=mybir.AluOpType.max,
                )

                evp = mpp2.tile([1, BS], F32, tag="ev")
                nc.tensor.matmul(
                    evp, mm_cast(w3t[:]), mm_cast(h2[:]), start=True, stop=True
                )
                ev = mlp.tile([1, BS], F32)
                nc.vector.tensor_scalar(
                    out=ev[:], in0=evp[:], scalar1=b3_sb[:, :], scalar2=None,
                    op0=mybir.AluOpType.add,
                )
                evs = mlp.tile([1, BS], F32)
                nc.vector.tensor_mul(out=evs[:], in0=ev[:], in1=stm_sb[:])
                nc.sync.dma_start(out=out[:, :], in_=evs[:])

    _split_multi_waits(nc)
    return nc


_NC_CACHE: dict = {}


def _get_nc(mm_f32r: bool = True, tr_f32r: bool = True) -> bass.Bass:
    key = (mm_f32r, tr_f32r)
    if key not in _NC_CACHE:
        _NC_CACHE[key] = build_kernel(mm_f32r=mm_f32r, tr_f32r=tr_f32r)
    return _NC_CACHE[key]


def make_in_maps(inputs: dict) -> list:
    """Shard full inputs into per-core input maps."""
    wf = np.ascontiguousarray(inputs["white_features"], dtype=np.float32)
    bf = np.ascontiguousarray(inputs["black_features"], dtype=np.float32)
    stm = np.ascontiguousarray(inputs["side_to_move"], dtype=np.float32)
    w_ftT = np.asarray(inputs["W_ft"], dtype=np.float32).T  # [IN, H]
    w_ftTs = np.ascontiguousarray(
        w_ftT.reshape(NSLAB, KT, 128, H).transpose(0, 2, 1, 3)
    ).reshape(NSLAB * 128, KT * H)
    w1T = np.asarray(inputs["W1"], dtype=np.float32).T  # [512, 32]
    w1Ts = np.ascontiguousarray(
        w1T.reshape(4, 128, 32).transpose(1, 0, 2)).reshape(128, 128)
    w2T = np.ascontiguousarray(np.asarray(inputs["W2"], dtype=np.float32).T)
    w3T = np.ascontiguousarray(np.asarray(inputs["W3"], dtype=np.float32).T)
    maps = []
    for c in range(N_CORES):
        sl = slice(c * BS, (c + 1) * BS)
        maps.append({
            "white_features": wf[sl],
            "black_features": bf[sl],
            "side_to_move": stm[sl].reshape(1, BS),
            "W_ftTs": w_ftTs,
            "W1Ts": w1Ts,
            "b1": np.ascontiguousarray(inputs["b1"], dtype=np.float32).reshape(32, 1),
            "W2T": w2T,
            "b2": np.ascontiguousarray(inputs["b2"], dtype=np.float32).reshape(32, 1),
            "W3T": w3T,
            "b3": np.ascontiguousarray(inputs["b3"], dtype=np.float32).reshape(1, 1),
        })
    return maps


def run(inputs: dict, trace: bool = False, mm_f32r: bool = True,
        tr_f32r: bool = True):
    """Run on all 8 cores; returns (full_output [4096,1] fp32, BassKernelResults)."""
    from concourse.bass_utils import run_bass_kernel_spmd

    nc = _get_nc(mm_f32r=mm_f32r, tr_f32r=tr_f32r)
    res = run_bass_kernel_spmd(
        nc, make_in_maps(inputs), core_ids=list(range(N_CORES)), trace=trace
    )
    full = np.concatenate(
        [res.results[c]["evaluation"].reshape(BS, 1) for c in range(N_CORES)],
        axis=0,
    ).astype(np.float32)
    return full, res


def kernel(**inputs) -> np.ndarray:
    return run(inputs, trace=False)[0]


if __name__ == "__main__":
    rng = np.random.default_rng(0)
    ins = {
        "white_features": rng.random((B, IN), dtype=np.float32),
        "black_features": rng.random((B, IN), dtype=np.float32),
        "side_to_move": np.ones((B,), dtype=np.float32),
        "W_ft": (0.1 * rng.standard_normal((H, IN))).astype(np.float32),
        "W1": (0.06 * rng.standard_normal((32, 2 * H))).astype(np.float32),
        "b1": np.zeros(32, np.float32),
        "W2": (0.17 * rng.standard_normal((32, 32))).astype(np.float32),
        "b2": np.zeros(32, np.float32),
        "W3": (0.24 * rng.standard_normal((1, 32))).astype(np.float32),
        "b3": np.zeros(1, np.float32),
    }
    out = kernel(**ins)
    # host reference
    whr = np.clip(ins["white_features"] @ ins["W_ft"].T, 0, 1)
    bhr = np.clip(ins["black_features"] @ ins["W_ft"].T, 0, 1)
    x = np.concatenate([whr, bhr], axis=1)
    x = np.maximum(x @ ins["W1"].T + ins["b1"], 0)
    x = np.maximum(x @ ins["W2"].T + ins["b2"], 0)
    ref = (x @ ins["W3"].T + ins["b3"]) * ins["side_to_move"][:, None]
    rel = np.linalg.norm(out - ref) / np.linalg.norm(ref)
    print("rel err:", rel)



# revision 3
# speedup vs baseline: 1.8175x; 1.8175x over previous
"""NNUE evaluation kernel for Trainium2 (8 NeuronCores, data-parallel batch).

reference math:
    wh = clip(white @ W_ft.T, 0, 1)        # [B, 256]
    bh = clip(black @ W_ft.T, 0, 1)        # [B, 256]
    x  = concat(wh, bh)                    # [B, 512]
    x  = relu(x @ W1.T + b1); x = relu(x @ W2.T + b2)
    ev = (x @ W3.T + b3) * stm[:, None]    # [B, 1]

Strategy: shard B=4096 across 8 cores (512 rows each), data-parallel, no
collectives. All GEMM operands are cast to fp16 on the host (the 2e-2
rel-err budget has ~30x margin at 16-bit; fp8 fails at ~1e-1) and the
features are host-transposed into the [k, b] layout the PE wants, so the
kernel is pure matmul: no on-chip transposes. Per core the two
[512, 40960] feature GEMMs read ~84 MB of fp16 features + 21 MB of fp16
W_ft from HBM (~295 us at 358 GB/s) and cost 1280 N=512 matmuls on the
PE (~273 us at 78.6 TF/s bf16-class rate) -- the kernel is balanced at
the joint roofline. Features/weights stream in k-slabs of 1024 (8
k-tiles), double buffered; W_ft.T tiles are stationary and feat.T
[k, b=512] moving, accumulating out.T [h, b] in PSUM across all 320
k-tiles (4 banks: 2 sides x 2 h-tiles). The clip is fused into the PSUM
evacuation and the tiny MLP stays in transposed [features, batch] layout
to the end.

This walrus build rejects instructions with >1 sync wait, so a post-pass
(_split_multi_waits) redistributes Tile-emitted waits onto single-wait
no-ops.
"""

import sys
import types

import numpy as np


def _inject_ntff_hook():
    """Register the axon NTFF profile hook if this image's antenv lacks it."""
    try:
        import antenv.axon_hooks  # noqa: F401
        return
    except ImportError:
        pass
    try:
        import trn_agent_boot.trn_boot as tb
        hook = tb._ntff_profile_via_ctypes("/opt/axon/libaxon_pjrt.so")
    except Exception:
        hook = None
    mod = types.ModuleType("antenv.axon_hooks")
    mod.get_axon_ntff_profile_hook = lambda: hook
    mod.set_axon_ntff_profile_hook = lambda h: None
    sys.modules["antenv.axon_hooks"] = mod


_inject_ntff_hook()

import concourse.bass as bass
import concourse.mybir as mybir
from concourse.tile import TileContext

N_CORES = 8
B = 4096
BS = B // N_CORES          # 512 batch rows per core
IN = 40960                 # feature count (contraction dim)
H = 256                    # hidden per perspective
NKT = 8                    # k-tiles per slab
KC = NKT * 128             # k-slab width: 1024
NSLAB = IN // KC           # 40
NKTOT = IN // 128          # 320 k-tiles total

F32 = mybir.dt.float32
F16 = mybir.dt.float16


def _split_multi_waits(nc: bass.Bass) -> None:
    """This walrus build rejects instructions carrying more than one sync
    wait. Split any such instruction: emit single-wait no-ops on the same
    engine immediately before it (same engine stream => same semantics)."""
    for f in nc.m.functions:
        for bb in f.blocks:
            new_insts = []
            changed = False
            for inst in bb.instructions:
                si = inst.sync_info
                waits = list(si.on_wait) if si is not None and si.on_wait else []
                if len(waits) > 1:
                    changed = True
                    for i, w in enumerate(waits[:-1]):
                        nop = mybir.InstNoOp(
                            name=f"{inst.name}-sw{i}", ins=[], outs=[]
                        )
                        nop.engine = inst.engine
                        nop.sync_info = mybir.SyncInfo(on_wait=[w], on_update=[])
                        nc.register_instruction(nop)
                        new_insts.append(nop)
                    inst.sync_info = mybir.SyncInfo(
                        on_wait=[waits[-1]],
                        on_update=list(si.on_update) if si.on_update else [],
                    )
                new_insts.append(inst)
            if changed:
                bb.instructions = new_insts


def build_kernel(mm_f32r: bool = True, tr_f32r: bool = True) -> bass.Bass:
    nc = bass.Bass()

    # Features arrive host-transposed+swizzled fp16: [128, NKTOT*BS] where
    # row p, columns [kt*BS : (kt+1)*BS] hold feat.T[kt*128 + p, :]. Each
    # k-slab DMA reads NKT*BS*2 = 8 KB contiguous per partition.
    wf = nc.dram_tensor("white_fT", [128, NKTOT * BS], F16, kind="ExternalInput")
    bf = nc.dram_tensor("black_fT", [128, NKTOT * BS], F16, kind="ExternalInput")
    # W_ft.T swizzled the same way: [128, NKTOT*H], 4 KB/partition per slab.
    w_ftTs = nc.dram_tensor("W_ftTs", [128, NKTOT * H], F16, kind="ExternalInput")
    w1Ts = nc.dram_tensor("W1Ts", [128, 128], F16, kind="ExternalInput")
    b1 = nc.dram_tensor("b1", [32, 1], F32, kind="ExternalInput")
    w2T = nc.dram_tensor("W2T", [32, 32], F16, kind="ExternalInput")
    b2 = nc.dram_tensor("b2", [32, 1], F32, kind="ExternalInput")
    w3T = nc.dram_tensor("W3T", [32, 1], F16, kind="ExternalInput")
    b3 = nc.dram_tensor("b3", [1, 1], F32, kind="ExternalInput")
    stm = nc.dram_tensor("side_to_move", [1, BS], F32, kind="ExternalInput")
    out = nc.dram_tensor("evaluation", [1, BS], F32, kind="ExternalOutput")

    feats = [wf, bf]

    with TileContext(nc) as tc:
        with (
            tc.tile_pool(name="ot_psum", bufs=1, space="PSUM") as ot_pool,
            tc.tile_pool(name="mlp", bufs=1) as mlp,
        ):
            # out.T accumulators: [h-tile 128, b 512] x (2 sides x 2 h-tiles)
            ot = [
                ot_pool.tile([128, BS], F32, tag=f"ot{i}", name=f"ot{i}")
                for i in range(4)
            ]

            # ---- main loop: feature-transformer GEMMs ----
            with (
                tc.tile_pool(name="fslab", bufs=2) as fslab_pool,
                tc.tile_pool(name="wt", bufs=2) as wt_pool,
            ):
                for s in range(NSLAB):
                    wt = wt_pool.tile([128, NKT, H], F16, tag="wt")
                    nc.scalar.dma_start(
                        out=wt[:],
                        in_=w_ftTs[:, s * NKT * H:(s + 1) * NKT * H],
                    )
                    fsl = []
                    for side in range(2):
                        f_t = fslab_pool.tile(
                            [128, NKT, BS], F16, tag=f"fslab{side}",
                            name=f"fsl{side}",
                        )
                        nc.sync.dma_start(
                            out=f_t[:],
                            in_=feats[side][
                                :, s * NKT * BS:(s + 1) * NKT * BS
                            ],
                        )
                        fsl.append(f_t)

                    for kt in range(NKT):
                        first = s == 0 and kt == 0
                        last = s == NSLAB - 1 and kt == NKT - 1
                        for h in range(2):
                            for side in range(2):
                                nc.tensor.matmul(
                                    ot[side * 2 + h],
                                    wt[:, kt, h * 128:(h + 1) * 128],
                                    fsl[side][:, kt, :],
                                    start=first,
                                    stop=last,
                                )

            # ---- MLP weight prep (emitted late so these DMAs schedule
            # behind the feature stream, not ahead of it) ----
            w1t = mlp.tile([128, 4, 32], F16)
            nc.scalar.dma_start(out=w1t[:], in_=w1Ts[:, :])
            w2t = mlp.tile([32, 32], F16)
            nc.scalar.dma_start(out=w2t[:], in_=w2T[:, :])
            w3t = mlp.tile([32, 1], F16)
            nc.scalar.dma_start(out=w3t[:], in_=w3T[:, :])
            b1_sb = mlp.tile([32, 1], F32)
            nc.scalar.dma_start(out=b1_sb[:], in_=b1[:, :])
            b2_sb = mlp.tile([32, 1], F32)
            nc.scalar.dma_start(out=b2_sb[:], in_=b2[:, :])
            b3_sb = mlp.tile([1, 1], F32)
            nc.scalar.dma_start(out=b3_sb[:], in_=b3[:, :])
            stm_sb = mlp.tile([1, BS], F32)
            nc.scalar.dma_start(out=stm_sb[:], in_=stm[:, :])

            # ---- clip + MLP (transposed layout throughout) ----
            with tc.tile_pool(name="mlp2_psum", bufs=1, space="PSUM") as mpp2:
                xt = []
                for i in range(4):
                    t = mlp.tile([128, BS], F16, tag=f"xt{i}")
                    nc.vector.tensor_scalar(
                        out=t[:], in0=ot[i][:], scalar1=0.0, scalar2=1.0,
                        op0=mybir.AluOpType.max, op1=mybir.AluOpType.min,
                    )
                    xt.append(t)

                h1p = mpp2.tile([32, BS], F32, tag="h1")
                for kt in range(4):
                    nc.tensor.matmul(
                        h1p, w1t[:, kt, :], xt[kt][:],
                        start=kt == 0, stop=kt == 3,
                    )
                h1 = mlp.tile([32, BS], F16)
                nc.vector.tensor_scalar(
                    out=h1[:], in0=h1p[:], scalar1=b1_sb[:, :], scalar2=0.0,
                    op0=mybir.AluOpType.add, op1=mybir.AluOpType.max,
                )

                h2p = mpp2.tile([32, BS], F32, tag="h2")
                nc.tensor.matmul(
                    h2p, w2t[:], h1[:], start=True, stop=True
                )
                h2 = mlp.tile([32, BS], F16)
                nc.vector.tensor_scalar(
                    out=h2[:], in0=h2p[:], scalar1=b2_sb[:, :], scalar2=0.0,
                    op0=mybir.AluOpType.add, op1=mybir.AluOpType.max,
                )

                evp = mpp2.tile([1, BS], F32, tag="ev")
                nc.tensor.matmul(
                    evp, w3t[:], h2[:], start=True, stop=True
                )
                ev = mlp.tile([1, BS], F32)
                nc.vector.tensor_scalar(
                    out=ev[:], in0=evp[:], scalar1=b3_sb[:, :], scalar2=None,
                    op0=mybir.AluOpType.add,
                )
                evs = mlp.tile([1, BS], F32)
                nc.vector.tensor_mul(out=evs[:], in0=ev[:], in1=stm_sb[:])
                nc.sync.dma_start(out=out[:, :], in_=evs[:])

    _split_multi_waits(nc)
    return nc


_NC_CACHE: dict = {}


def _get_nc(mm_f32r: bool = True, tr_f32r: bool = True) -> bass.Bass:
    key = (mm_f32r, tr_f32r)
    if key not in _NC_CACHE:
        _NC_CACHE[key] = build_kernel(mm_f32r=mm_f32r, tr_f32r=tr_f32r)
    return _NC_CACHE[key]


def _swizzle_T(arr_f16: np.ndarray, ncols: int) -> np.ndarray:
    """[rows, IN] fp16 -> [128, NKTOT*rows] where row p, cols
    [kt*rows:(kt+1)*rows] = arr.T[kt*128 + p, :]."""
    rows = arr_f16.shape[0]
    assert arr_f16.shape == (rows, IN) and ncols == rows
    return np.ascontiguousarray(
        arr_f16.reshape(rows, NKTOT, 128).transpose(2, 1, 0)
    ).reshape(128, NKTOT * rows)


def make_in_maps(inputs: dict) -> list:
    """Shard full inputs into per-core input maps (fp16, transposed)."""
    wf = np.asarray(inputs["white_features"]).astype(np.float16)
    bf = np.asarray(inputs["black_features"]).astype(np.float16)
    stm = np.ascontiguousarray(inputs["side_to_move"], dtype=np.float32)
    w_ftTs = _swizzle_T(
        np.asarray(inputs["W_ft"], dtype=np.float32).astype(np.float16), H)
    w1T = np.asarray(inputs["W1"], dtype=np.float32).astype(np.float16).T
    w1Ts = np.ascontiguousarray(
        w1T.reshape(4, 128, 32).transpose(1, 0, 2)).reshape(128, 128)
    w2T = np.ascontiguousarray(
        np.asarray(inputs["W2"], dtype=np.float32).astype(np.float16).T)
    w3T = np.ascontiguousarray(
        np.asarray(inputs["W3"], dtype=np.float32).astype(np.float16).T)
    maps = []
    for c in range(N_CORES):
        sl = slice(c * BS, (c + 1) * BS)
        maps.append({
            "white_fT": _swizzle_T(wf[sl], BS),
            "black_fT": _swizzle_T(bf[sl], BS),
            "side_to_move": stm[sl].reshape(1, BS),
            "W_ftTs": w_ftTs,
            "W1Ts": w1Ts,
            "b1": np.ascontiguousarray(inputs["b1"], dtype=np.float32).reshape(32, 1),
            "W2T": w2T,
            "b2": np.ascontiguousarray(inputs["b2"], dtype=np.float32).reshape(32, 1),
            "W3T": w3T,
            "b3": np.ascontiguousarray(inputs["b3"], dtype=np.float32).reshape(1, 1),
        })
    return maps


def run(inputs: dict, trace: bool = False, mm_f32r: bool = True,
        tr_f32r: bool = True):
    """Run on all 8 cores; returns (full_output [4096,1] fp32, BassKernelResults)."""
    from concourse.bass_utils import run_bass_kernel_spmd

    nc = _get_nc(mm_f32r=mm_f32r, tr_f32r=tr_f32r)
    res = run_bass_kernel_spmd(
        nc, make_in_maps(inputs), core_ids=list(range(N_CORES)), trace=trace
    )
    full = np.concatenate(
        [res.results[c]["evaluation"].reshape(BS, 1) for c in range(N_CORES)],
        axis=0,
    ).astype(np.float32)
    return full, res


def kernel(**inputs) -> np.ndarray:
    return run(inputs, trace=False)[0]


if __name__ == "__main__":
    rng = np.random.default_rng(0)
    ins = {
        "white_features": rng.random((B, IN), dtype=np.float32),
        "black_features": rng.random((B, IN), dtype=np.float32),
        "side_to_move": np.ones((B,), dtype=np.float32),
        "W_ft": (0.1 * rng.standard_normal((H, IN))).astype(np.float32),
        "W1": (0.06 * rng.standard_normal((32, 2 * H))).astype(np.float32),
        "b1": np.zeros(32, np.float32),
        "W2": (0.17 * rng.standard_normal((32, 32))).astype(np.float32),
        "b2": np.zeros(32, np.float32),
        "W3": (0.24 * rng.standard_normal((1, 32))).astype(np.float32),
        "b3": np.zeros(1, np.float32),
    }
    out = kernel(**ins)
    # host reference
    whr = np.clip(ins["white_features"] @ ins["W_ft"].T, 0, 1)
    bhr = np.clip(ins["black_features"] @ ins["W_ft"].T, 0, 1)
    x = np.concatenate([whr, bhr], axis=1)
    x = np.maximum(x @ ins["W1"].T + ins["b1"], 0)
    x = np.maximum(x @ ins["W2"].T + ins["b2"], 0)
    ref = (x @ ins["W3"].T + ins["b3"]) * ins["side_to_move"][:, None]
    rel = np.linalg.norm(out - ref) / np.linalg.norm(ref)
    print("rel err:", rel)


# revision 5
# speedup vs baseline: 2.0294x; 1.1166x over previous
"""NNUE evaluation kernel for Trainium2 (8 NeuronCores, data-parallel batch).

reference math:
    wh = clip(white @ W_ft.T, 0, 1)        # [B, 256]
    bh = clip(black @ W_ft.T, 0, 1)        # [B, 256]
    x  = concat(wh, bh)                    # [B, 512]
    x  = relu(x @ W1.T + b1); x = relu(x @ W2.T + b2)
    ev = (x @ W3.T + b3) * stm[:, None]    # [B, 1]

Strategy: shard B=4096 across 8 cores (512 rows each), data-parallel, no
collectives. All GEMM operands are cast to fp16 on the host (the 2e-2
rel-err budget has ~30x margin at 16-bit; fp8 fails at ~1e-1) and the
features are host-transposed into the [k, b] layout the PE wants, so the
kernel is pure matmul: no on-chip transposes. Per core the two
[512, 40960] feature GEMMs read ~84 MB of fp16 features + 21 MB of fp16
W_ft from HBM (~295 us at 358 GB/s) and cost 1280 N=512 matmuls on the
PE (~273 us at 78.6 TF/s bf16-class rate) -- the kernel is balanced at
the joint roofline. Features/weights stream in k-slabs of 1024 (8
k-tiles), double buffered; W_ft.T tiles are stationary and feat.T
[k, b=512] moving, accumulating out.T [h, b] in PSUM across all 320
k-tiles (4 banks: 2 sides x 2 h-tiles). The clip is fused into the PSUM
evacuation and the tiny MLP stays in transposed [features, batch] layout
to the end.

This walrus build rejects instructions with >1 sync wait, so a post-pass
(_split_multi_waits) redistributes Tile-emitted waits onto single-wait
no-ops.
"""

import sys
import types

import numpy as np


def _inject_ntff_hook():
    """Register the axon NTFF profile hook if this image's antenv lacks it."""
    try:
        import antenv.axon_hooks  # noqa: F401
        return
    except ImportError:
        pass
    try:
        import trn_agent_boot.trn_boot as tb
        hook = tb._ntff_profile_via_ctypes("/opt/axon/libaxon_pjrt.so")
    except Exception:
        hook = None
    mod = types.ModuleType("antenv.axon_hooks")
    mod.get_axon_ntff_profile_hook = lambda: hook
    mod.set_axon_ntff_profile_hook = lambda h: None
    sys.modules["antenv.axon_hooks"] = mod


_inject_ntff_hook()

import concourse.bass as bass
import concourse.mybir as mybir
from concourse.tile import TileContext

N_CORES = 8
B = 4096
BS = B // N_CORES          # 512 batch rows per core
IN = 40960                 # feature count (contraction dim)
H = 256                    # hidden per perspective
NKT = 4                    # k-tiles per slab
KC = NKT * 128             # k-slab width: 512
NSLAB = IN // KC           # 80
NKTOT = IN // 128          # 320 k-tiles total

F32 = mybir.dt.float32
F16 = mybir.dt.float16


def _split_multi_waits(nc: bass.Bass) -> None:
    """This walrus build rejects instructions carrying more than one sync
    wait. Split any such instruction: emit single-wait no-ops on the same
    engine immediately before it (same engine stream => same semantics)."""
    for f in nc.m.functions:
        for bb in f.blocks:
            new_insts = []
            changed = False
            for inst in bb.instructions:
                si = inst.sync_info
                waits = list(si.on_wait) if si is not None and si.on_wait else []
                if len(waits) > 1:
                    changed = True
                    for i, w in enumerate(waits[:-1]):
                        nop = mybir.InstNoOp(
                            name=f"{inst.name}-sw{i}", ins=[], outs=[]
                        )
                        nop.engine = inst.engine
                        nop.sync_info = mybir.SyncInfo(on_wait=[w], on_update=[])
                        nc.register_instruction(nop)
                        new_insts.append(nop)
                    inst.sync_info = mybir.SyncInfo(
                        on_wait=[waits[-1]],
                        on_update=list(si.on_update) if si.on_update else [],
                    )
                new_insts.append(inst)
            if changed:
                bb.instructions = new_insts


def build_kernel(mm_f32r: bool = True, tr_f32r: bool = True) -> bass.Bass:
    nc = bass.Bass()

    # Features arrive host-transposed+swizzled fp16: [128, NKTOT*BS] where
    # row p, columns [kt*BS : (kt+1)*BS] hold feat.T[kt*128 + p, :]. Each
    # k-slab DMA reads NKT*BS*2 = 8 KB contiguous per partition.
    wf = nc.dram_tensor("white_fT", [128, NKTOT * BS], F16, kind="ExternalInput")
    bf = nc.dram_tensor("black_fT", [128, NKTOT * BS], F16, kind="ExternalInput")
    # W_ft.T swizzled the same way: [128, NKTOT*H], 4 KB/partition per slab.
    w_ftTs = nc.dram_tensor("W_ftTs", [128, NKTOT * H], F16, kind="ExternalInput")
    w1Ts = nc.dram_tensor("W1Ts", [128, 128], F16, kind="ExternalInput")
    b1 = nc.dram_tensor("b1", [32, 1], F32, kind="ExternalInput")
    w2T = nc.dram_tensor("W2T", [32, 32], F16, kind="ExternalInput")
    b2 = nc.dram_tensor("b2", [32, 1], F32, kind="ExternalInput")
    w3T = nc.dram_tensor("W3T", [32, 1], F16, kind="ExternalInput")
    b3 = nc.dram_tensor("b3", [1, 1], F32, kind="ExternalInput")
    stm = nc.dram_tensor("side_to_move", [1, BS], F32, kind="ExternalInput")
    out = nc.dram_tensor("evaluation", [1, BS], F32, kind="ExternalOutput")

    feats = [wf, bf]

    with TileContext(nc) as tc:
        with (
            tc.tile_pool(name="ot_psum", bufs=1, space="PSUM") as ot_pool,
            tc.tile_pool(name="mlp", bufs=1) as mlp,
        ):
            # out.T accumulators: [h-tile 128, b 512] x (2 sides x 2 h-tiles)
            ot = [
                ot_pool.tile([128, BS], F32, tag=f"ot{i}", name=f"ot{i}")
                for i in range(4)
            ]

            # ---- main loop: feature-transformer GEMMs ----
            with (
                tc.tile_pool(name="fslab", bufs=4) as fslab_pool,
                tc.tile_pool(name="wt", bufs=4) as wt_pool,
            ):
                # white features ride the sync HWDGE ring, black the
                # scalar ring; W_ft alternates so both rings carry ~52 MB.
                f_eng = [nc.sync, nc.scalar]
                for s in range(NSLAB):
                    wt = wt_pool.tile([128, NKT, H], F16, tag="wt")
                    f_eng[s % 2].dma_start(
                        out=wt[:],
                        in_=w_ftTs[:, s * NKT * H:(s + 1) * NKT * H],
                    )
                    fsl = []
                    for side in range(2):
                        f_t = fslab_pool.tile(
                            [128, NKT, BS], F16, tag=f"fslab{side}",
                            name=f"fsl{side}",
                        )
                        f_eng[side].dma_start(
                            out=f_t[:],
                            in_=feats[side][
                                :, s * NKT * BS:(s + 1) * NKT * BS
                            ],
                        )
                        fsl.append(f_t)

                    for kt in range(NKT):
                        first = s == 0 and kt == 0
                        last = s == NSLAB - 1 and kt == NKT - 1
                        for h in range(2):
                            for side in range(2):
                                nc.tensor.matmul(
                                    ot[side * 2 + h],
                                    wt[:, kt, h * 128:(h + 1) * 128],
                                    fsl[side][:, kt, :],
                                    start=first,
                                    stop=last,
                                )

            # ---- MLP weight prep (emitted late so these DMAs schedule
            # behind the feature stream, not ahead of it) ----
            w1t = mlp.tile([128, 4, 32], F16)
            nc.scalar.dma_start(out=w1t[:], in_=w1Ts[:, :])
            w2t = mlp.tile([32, 32], F16)
            nc.scalar.dma_start(out=w2t[:], in_=w2T[:, :])
            w3t = mlp.tile([32, 1], F16)
            nc.scalar.dma_start(out=w3t[:], in_=w3T[:, :])
            b1_sb = mlp.tile([32, 1], F32)
            nc.scalar.dma_start(out=b1_sb[:], in_=b1[:, :])
            b2_sb = mlp.tile([32, 1], F32)
            nc.scalar.dma_start(out=b2_sb[:], in_=b2[:, :])
            b3_sb = mlp.tile([1, 1], F32)
            nc.scalar.dma_start(out=b3_sb[:], in_=b3[:, :])
            stm_sb = mlp.tile([1, BS], F32)
            nc.scalar.dma_start(out=stm_sb[:], in_=stm[:, :])

            # ---- clip + MLP (transposed layout throughout) ----
            with tc.tile_pool(name="mlp2_psum", bufs=1, space="PSUM") as mpp2:
                xt = []
                for i in range(4):
                    t = mlp.tile([128, BS], F16, tag=f"xt{i}")
                    nc.vector.tensor_scalar(
                        out=t[:], in0=ot[i][:], scalar1=0.0, scalar2=1.0,
                        op0=mybir.AluOpType.max, op1=mybir.AluOpType.min,
                    )
                    xt.append(t)

                h1p = mpp2.tile([32, BS], F32, tag="h1")
                for kt in range(4):
                    nc.tensor.matmul(
                        h1p, w1t[:, kt, :], xt[kt][:],
                        start=kt == 0, stop=kt == 3,
                    )
                h1 = mlp.tile([32, BS], F16)
                nc.vector.tensor_scalar(
                    out=h1[:], in0=h1p[:], scalar1=b1_sb[:, :], scalar2=0.0,
                    op0=mybir.AluOpType.add, op1=mybir.AluOpType.max,
                )

                h2p = mpp2.tile([32, BS], F32, tag="h2")
                nc.tensor.matmul(
                    h2p, w2t[:], h1[:], start=True, stop=True
                )
                h2 = mlp.tile([32, BS], F16)
                nc.vector.tensor_scalar(
                    out=h2[:], in0=h2p[:], scalar1=b2_sb[:, :], scalar2=0.0,
                    op0=mybir.AluOpType.add, op1=mybir.AluOpType.max,
                )

                evp = mpp2.tile([1, BS], F32, tag="ev")
                nc.tensor.matmul(
                    evp, w3t[:], h2[:], start=True, stop=True
                )
                ev = mlp.tile([1, BS], F32)
                nc.vector.tensor_scalar(
                    out=ev[:], in0=evp[:], scalar1=b3_sb[:, :], scalar2=None,
                    op0=mybir.AluOpType.add,
                )
                evs = mlp.tile([1, BS], F32)
                nc.vector.tensor_mul(out=evs[:], in0=ev[:], in1=stm_sb[:])
                nc.sync.dma_start(out=out[:, :], in_=evs[:])

    _split_multi_waits(nc)
    return nc


_NC_CACHE: dict = {}


def _get_nc(mm_f32r: bool = True, tr_f32r: bool = True) -> bass.Bass:
    key = (mm_f32r, tr_f32r)
    if key not in _NC_CACHE:
        _NC_CACHE[key] = build_kernel(mm_f32r=mm_f32r, tr_f32r=tr_f32r)
    return _NC_CACHE[key]


def _swizzle_T(arr_f16: np.ndarray, ncols: int) -> np.ndarray:
    """[rows, IN] fp16 -> [128, NKTOT*rows] where row p, cols
    [kt*rows:(kt+1)*rows] = arr.T[kt*128 + p, :]."""
    rows = arr_f16.shape[0]
    assert arr_f16.shape == (rows, IN) and ncols == rows
    return np.ascontiguousarray(
        arr_f16.reshape(rows, NKTOT, 128).transpose(2, 1, 0)
    ).reshape(128, NKTOT * rows)


def make_in_maps(inputs: dict) -> list:
    """Shard full inputs into per-core input maps (fp16, transposed)."""
    wf = np.asarray(inputs["white_features"]).astype(np.float16)
    bf = np.asarray(inputs["black_features"]).astype(np.float16)
    stm = np.ascontiguousarray(inputs["side_to_move"], dtype=np.float32)
    w_ftTs = _swizzle_T(
        np.asarray(inputs["W_ft"], dtype=np.float32).astype(np.float16), H)
    w1T = np.asarray(inputs["W1"], dtype=np.float32).astype(np.float16).T
    w1Ts = np.ascontiguousarray(
        w1T.reshape(4, 128, 32).transpose(1, 0, 2)).reshape(128, 128)
    w2T = np.ascontiguousarray(
        np.asarray(inputs["W2"], dtype=np.float32).astype(np.float16).T)
    w3T = np.ascontiguousarray(
        np.asarray(inputs["W3"], dtype=np.float32).astype(np.float16).T)
    maps = []
    for c in range(N_CORES):
        sl = slice(c * BS, (c + 1) * BS)
        maps.append({
            "white_fT": _swizzle_T(wf[sl], BS),
            "black_fT": _swizzle_T(bf[sl], BS),
            "side_to_move": stm[sl].reshape(1, BS),
            "W_ftTs": w_ftTs,
            "W1Ts": w1Ts,
            "b1": np.ascontiguousarray(inputs["b1"], dtype=np.float32).reshape(32, 1),
            "W2T": w2T,
            "b2": np.ascontiguousarray(inputs["b2"], dtype=np.float32).reshape(32, 1),
            "W3T": w3T,
            "b3": np.ascontiguousarray(inputs["b3"], dtype=np.float32).reshape(1, 1),
        })
    return maps


def run(inputs: dict, trace: bool = False, mm_f32r: bool = True,
        tr_f32r: bool = True):
    """Run on all 8 cores; returns (full_output [4096,1] fp32, BassKernelResults)."""
    from concourse.bass_utils import run_bass_kernel_spmd

    nc = _get_nc(mm_f32r=mm_f32r, tr_f32r=tr_f32r)
    res = run_bass_kernel_spmd(
        nc, make_in_maps(inputs), core_ids=list(range(N_CORES)), trace=trace
    )
    full = np.concatenate(
        [res.results[c]["evaluation"].reshape(BS, 1) for c in range(N_CORES)],
        axis=0,
    ).astype(np.float32)
    return full, res


def kernel(**inputs) -> np.ndarray:
    return run(inputs, trace=False)[0]


if __name__ == "__main__":
    rng = np.random.default_rng(0)
    ins = {
        "white_features": rng.random((B, IN), dtype=np.float32),
        "black_features": rng.random((B, IN), dtype=np.float32),
        "side_to_move": np.ones((B,), dtype=np.float32),
        "W_ft": (0.1 * rng.standard_normal((H, IN))).astype(np.float32),
        "W1": (0.06 * rng.standard_normal((32, 2 * H))).astype(np.float32),
        "b1": np.zeros(32, np.float32),
        "W2": (0.17 * rng.standard_normal((32, 32))).astype(np.float32),
        "b2": np.zeros(32, np.float32),
        "W3": (0.24 * rng.standard_normal((1, 32))).astype(np.float32),
        "b3": np.zeros(1, np.float32),
    }
    out = kernel(**ins)
    # host reference
    whr = np.clip(ins["white_features"] @ ins["W_ft"].T, 0, 1)
    bhr = np.clip(ins["black_features"] @ ins["W_ft"].T, 0, 1)
    x = np.concatenate([whr, bhr], axis=1)
    x = np.maximum(x @ ins["W1"].T + ins["b1"], 0)
    x = np.maximum(x @ ins["W2"].T + ins["b2"], 0)
    ref = (x @ ins["W3"].T + ins["b3"]) * ins["side_to_move"][:, None]
    rel = np.linalg.norm(out - ref) / np.linalg.norm(ref)
    print("rel err:", rel)


# revision 8
# speedup vs baseline: 2.1512x; 1.0600x over previous
"""NNUE evaluation kernel for Trainium2 (8 NeuronCores, data-parallel batch).

reference math:
    wh = clip(white @ W_ft.T, 0, 1)        # [B, 256]
    bh = clip(black @ W_ft.T, 0, 1)        # [B, 256]
    x  = concat(wh, bh)                    # [B, 512]
    x  = relu(x @ W1.T + b1); x = relu(x @ W2.T + b2)
    ev = (x @ W3.T + b3) * stm[:, None]    # [B, 1]

Strategy: shard B=4096 across 8 cores (512 rows each), data-parallel, no
collectives. All GEMM operands are cast to fp16 on the host (the 2e-2
rel-err budget has ~30x margin at 16-bit; fp8 fails at ~1e-1) and the
features are host-transposed into the [k, b] layout the PE wants, so the
kernel is pure matmul: no on-chip transposes. Per core the two
[512, 40960] feature GEMMs read ~84 MB of fp16 features + 21 MB of fp16
W_ft from HBM (~295 us at 358 GB/s) and cost 1280 N=512 matmuls on the
PE (~273 us at 78.6 TF/s bf16-class rate) -- the kernel is balanced at
the joint roofline. Features/weights stream in k-slabs of 1024 (8
k-tiles), double buffered; W_ft.T tiles are stationary and feat.T
[k, b=512] moving, accumulating out.T [h, b] in PSUM across all 320
k-tiles (4 banks: 2 sides x 2 h-tiles). The clip is fused into the PSUM
evacuation and the tiny MLP stays in transposed [features, batch] layout
to the end.

This walrus build rejects instructions with >1 sync wait, so a post-pass
(_split_multi_waits) redistributes Tile-emitted waits onto single-wait
no-ops.
"""

import sys
import types

import numpy as np


def _inject_ntff_hook():
    """Register the axon NTFF profile hook if this image's antenv lacks it."""
    try:
        import antenv.axon_hooks  # noqa: F401
        return
    except ImportError:
        pass
    try:
        import trn_agent_boot.trn_boot as tb
        hook = tb._ntff_profile_via_ctypes("/opt/axon/libaxon_pjrt.so")
    except Exception:
        hook = None
    mod = types.ModuleType("antenv.axon_hooks")
    mod.get_axon_ntff_profile_hook = lambda: hook
    mod.set_axon_ntff_profile_hook = lambda h: None
    sys.modules["antenv.axon_hooks"] = mod


_inject_ntff_hook()

import concourse.bass as bass
import concourse.mybir as mybir
from concourse.tile import TileContext

N_CORES = 8
B = 4096
BS = B // N_CORES          # 512 batch rows per core
IN = 40960                 # feature count (contraction dim)
H = 256                    # hidden per perspective
NKT = 4                    # k-tiles per slab
KC = NKT * 128             # k-slab width: 512
NSLAB = IN // KC           # 80
NKTOT = IN // 128          # 320 k-tiles total

F32 = mybir.dt.float32
F16 = mybir.dt.float16


def _split_multi_waits(nc: bass.Bass) -> None:
    """This walrus build rejects instructions carrying more than one sync
    wait. Split any such instruction: emit single-wait no-ops on the same
    engine immediately before it (same engine stream => same semantics)."""
    for f in nc.m.functions:
        for bb in f.blocks:
            new_insts = []
            changed = False
            for inst in bb.instructions:
                si = inst.sync_info
                waits = list(si.on_wait) if si is not None and si.on_wait else []
                if len(waits) > 1:
                    changed = True
                    for i, w in enumerate(waits[:-1]):
                        nop = mybir.InstNoOp(
                            name=f"{inst.name}-sw{i}", ins=[], outs=[]
                        )
                        nop.engine = inst.engine
                        nop.sync_info = mybir.SyncInfo(on_wait=[w], on_update=[])
                        nc.register_instruction(nop)
                        new_insts.append(nop)
                    inst.sync_info = mybir.SyncInfo(
                        on_wait=[waits[-1]],
                        on_update=list(si.on_update) if si.on_update else [],
                    )
                new_insts.append(inst)
            if changed:
                bb.instructions = new_insts


def build_kernel(mm_f32r: bool = True, tr_f32r: bool = True) -> bass.Bass:
    nc = bass.Bass()

    # Features arrive host-transposed+swizzled fp16: [128, NKTOT*BS] where
    # row p, columns [kt*BS : (kt+1)*BS] hold feat.T[kt*128 + p, :]. Each
    # k-slab DMA reads NKT*BS*2 = 8 KB contiguous per partition.
    wf = nc.dram_tensor("white_fT", [128, NKTOT * BS], F16, kind="ExternalInput")
    bf = nc.dram_tensor("black_fT", [128, NKTOT * BS], F16, kind="ExternalInput")
    # W_ft.T swizzled the same way: [128, NKTOT*H], 4 KB/partition per slab.
    w_ftTs = nc.dram_tensor("W_ftTs", [128, NKTOT * H], F16, kind="ExternalInput")
    w1Ts = nc.dram_tensor("W1Ts", [128, 128], F16, kind="ExternalInput")
    b1 = nc.dram_tensor("b1", [32, 1], F32, kind="ExternalInput")
    w2T = nc.dram_tensor("W2T", [32, 32], F16, kind="ExternalInput")
    b2 = nc.dram_tensor("b2", [32, 1], F32, kind="ExternalInput")
    w3T = nc.dram_tensor("W3T", [32, 1], F16, kind="ExternalInput")
    b3 = nc.dram_tensor("b3", [1, 1], F32, kind="ExternalInput")
    stm = nc.dram_tensor("side_to_move", [1, BS], F32, kind="ExternalInput")
    out = nc.dram_tensor("evaluation", [1, BS], F32, kind="ExternalOutput")

    feats = [wf, bf]

    with TileContext(nc) as tc:
        with (
            tc.tile_pool(name="ot_psum", bufs=1, space="PSUM") as ot_pool,
            tc.tile_pool(name="mlp", bufs=1) as mlp,
        ):
            # out.T accumulators: [h-tile 128, b 512] x (2 sides x 2 h-tiles)
            ot = [
                ot_pool.tile([128, BS], F32, tag=f"ot{i}", name=f"ot{i}")
                for i in range(4)
            ]
            xt = []  # clipped fp16 copies, filled during the last slab

            # ---- main loop: feature-transformer GEMMs ----
            # slab widths in k-tiles: two small warmup slabs so the PE
            # starts early, then uniform NKT-wide slabs.
            widths = [2, 2] + [NKT] * ((NKTOT - 4) // NKT)
            assert sum(widths) == NKTOT
            with (
                tc.tile_pool(name="fslab", bufs=8) as fslab_pool,
                tc.tile_pool(name="wt", bufs=8) as wt_pool,
                tc.tile_pool(name="pre", bufs=1) as pre_pool,
            ):
                # white features ride the sync HWDGE ring, black the
                # scalar ring; W_ft alternates so both rings carry ~52 MB.
                f_eng = [nc.sync, nc.scalar]
                kt0 = 0
                for s, w in enumerate(widths):
                    pre = w != NKT
                    pool = pre_pool if pre else fslab_pool
                    wpool = pre_pool if pre else wt_pool
                    wt = wpool.tile([128, w, H], F16, tag=f"wt{s}" if pre else "wt",
                                    name="wt")
                    f_eng[s % 2].dma_start(
                        out=wt[:],
                        in_=w_ftTs[:, kt0 * H:(kt0 + w) * H],
                    )
                    fsl = []
                    for side in range(2):
                        f_t = pool.tile(
                            [128, w, BS], F16,
                            tag=f"pre{side}_{s}" if pre else f"fslab{side}",
                            name=f"fsl{side}",
                        )
                        f_eng[side].dma_start(
                            out=f_t[:],
                            in_=feats[side][
                                :, kt0 * BS:(kt0 + w) * BS
                            ],
                        )
                        fsl.append(f_t)

                    last_slab = s == len(widths) - 1
                    if not last_slab:
                        for kt in range(w):
                            first = kt0 == 0 and kt == 0
                            for h in range(2):
                                for side in range(2):
                                    nc.tensor.matmul(
                                        ot[side * 2 + h],
                                        wt[:, kt, h * 128:(h + 1) * 128],
                                        fsl[side][:, kt, :],
                                        start=first,
                                        stop=False,
                                    )
                    else:
                        # final slab: finish white first, evacuate its
                        # PSUM banks while black's last matmuls run.
                        for side in range(2):
                            for kt in range(w):
                                for h in range(2):
                                    nc.tensor.matmul(
                                        ot[side * 2 + h],
                                        wt[:, kt, h * 128:(h + 1) * 128],
                                        fsl[side][:, kt, :],
                                        start=False,
                                        stop=kt == w - 1,
                                    )
                            for i in range(2 * side, 2 * side + 2):
                                t = mlp.tile([128, BS], F16, tag=f"xt{i}",
                                             name="xt")
                                nc.vector.tensor_scalar(
                                    out=t[:], in0=ot[i][:],
                                    scalar1=0.0, scalar2=1.0,
                                    op0=mybir.AluOpType.max,
                                    op1=mybir.AluOpType.min,
                                )
                                xt.append(t)
                    kt0 += w

            # ---- MLP weight prep (emitted late so these DMAs schedule
            # behind the feature stream, not ahead of it) ----
            w1t = mlp.tile([128, 4, 32], F16)
            nc.scalar.dma_start(out=w1t[:], in_=w1Ts[:, :])
            w2t = mlp.tile([32, 32], F16)
            nc.scalar.dma_start(out=w2t[:], in_=w2T[:, :])
            w3t = mlp.tile([32, 1], F16)
            nc.scalar.dma_start(out=w3t[:], in_=w3T[:, :])
            b1_sb = mlp.tile([32, 1], F32)
            nc.scalar.dma_start(out=b1_sb[:], in_=b1[:, :])
            b2_sb = mlp.tile([32, 1], F32)
            nc.scalar.dma_start(out=b2_sb[:], in_=b2[:, :])
            b3_sb = mlp.tile([1, 1], F32)
            nc.scalar.dma_start(out=b3_sb[:], in_=b3[:, :])
            stm_sb = mlp.tile([1, BS], F32)
            nc.scalar.dma_start(out=stm_sb[:], in_=stm[:, :])

            # ---- MLP (transposed layout throughout; xt built above) ----
            with tc.tile_pool(name="mlp2_psum", bufs=1, space="PSUM") as mpp2:
                h1p = mpp2.tile([32, BS], F32, tag="h1")
                for kt in range(4):
                    nc.tensor.matmul(
                        h1p, w1t[:, kt, :], xt[kt][:],
                        start=kt == 0, stop=kt == 3,
                    )
                h1 = mlp.tile([32, BS], F16)
                nc.vector.tensor_scalar(
                    out=h1[:], in0=h1p[:], scalar1=b1_sb[:, :], scalar2=0.0,
                    op0=mybir.AluOpType.add, op1=mybir.AluOpType.max,
                )

                h2p = mpp2.tile([32, BS], F32, tag="h2")
                nc.tensor.matmul(
                    h2p, w2t[:], h1[:], start=True, stop=True
                )
                h2 = mlp.tile([32, BS], F16)
                nc.vector.tensor_scalar(
                    out=h2[:], in0=h2p[:], scalar1=b2_sb[:, :], scalar2=0.0,
                    op0=mybir.AluOpType.add, op1=mybir.AluOpType.max,
                )

                evp = mpp2.tile([1, BS], F32, tag="ev")
                nc.tensor.matmul(
                    evp, w3t[:], h2[:], start=True, stop=True
                )
                ev = mlp.tile([1, BS], F32)
                nc.vector.tensor_scalar(
                    out=ev[:], in0=evp[:], scalar1=b3_sb[:, :], scalar2=None,
                    op0=mybir.AluOpType.add,
                )
                evs = mlp.tile([1, BS], F32)
                nc.vector.tensor_mul(out=evs[:], in0=ev[:], in1=stm_sb[:])
                nc.sync.dma_start(out=out[:, :], in_=evs[:])

    _split_multi_waits(nc)
    return nc


_NC_CACHE: dict = {}


def _get_nc(mm_f32r: bool = True, tr_f32r: bool = True) -> bass.Bass:
    key = (mm_f32r, tr_f32r)
    if key not in _NC_CACHE:
        _NC_CACHE[key] = build_kernel(mm_f32r=mm_f32r, tr_f32r=tr_f32r)
    return _NC_CACHE[key]


def _swizzle_T(arr_f16: np.ndarray, ncols: int) -> np.ndarray:
    """[rows, IN] fp16 -> [128, NKTOT*rows] where row p, cols
    [kt*rows:(kt+1)*rows] = arr.T[kt*128 + p, :]."""
    rows = arr_f16.shape[0]
    assert arr_f16.shape == (rows, IN) and ncols == rows
    return np.ascontiguousarray(
        arr_f16.reshape(rows, NKTOT, 128).transpose(2, 1, 0)
    ).reshape(128, NKTOT * rows)


def make_in_maps(inputs: dict) -> list:
    """Shard full inputs into per-core input maps (fp16, transposed)."""
    wf = np.asarray(inputs["white_features"]).astype(np.float16)
    bf = np.asarray(inputs["black_features"]).astype(np.float16)
    stm = np.ascontiguousarray(inputs["side_to_move"], dtype=np.float32)
    w_ftTs = _swizzle_T(
        np.asarray(inputs["W_ft"], dtype=np.float32).astype(np.float16), H)
    w1T = np.asarray(inputs["W1"], dtype=np.float32).astype(np.float16).T
    w1Ts = np.ascontiguousarray(
        w1T.reshape(4, 128, 32).transpose(1, 0, 2)).reshape(128, 128)
    w2T = np.ascontiguousarray(
        np.asarray(inputs["W2"], dtype=np.float32).astype(np.float16).T)
    w3T = np.ascontiguousarray(
        np.asarray(inputs["W3"], dtype=np.float32).astype(np.float16).T)
    maps = []
    for c in range(N_CORES):
        sl = slice(c * BS, (c + 1) * BS)
        maps.append({
            "white_fT": _swizzle_T(wf[sl], BS),
            "black_fT": _swizzle_T(bf[sl], BS),
            "side_to_move": stm[sl].reshape(1, BS),
            "W_ftTs": w_ftTs,
            "W1Ts": w1Ts,
            "b1": np.ascontiguousarray(inputs["b1"], dtype=np.float32).reshape(32, 1),
            "W2T": w2T,
            "b2": np.ascontiguousarray(inputs["b2"], dtype=np.float32).reshape(32, 1),
            "W3T": w3T,
            "b3": np.ascontiguousarray(inputs["b3"], dtype=np.float32).reshape(1, 1),
        })
    return maps


def run(inputs: dict, trace: bool = False, mm_f32r: bool = True,
        tr_f32r: bool = True):
    """Run on all 8 cores; returns (full_output [4096,1] fp32, BassKernelResults)."""
    from concourse.bass_utils import run_bass_kernel_spmd

    nc = _get_nc(mm_f32r=mm_f32r, tr_f32r=tr_f32r)
    res = run_bass_kernel_spmd(
        nc, make_in_maps(inputs), core_ids=list(range(N_CORES)), trace=trace
    )
    full = np.concatenate(
        [res.results[c]["evaluation"].reshape(BS, 1) for c in range(N_CORES)],
        axis=0,
    ).astype(np.float32)
    return full, res


def kernel(**inputs) -> np.ndarray:
    return run(inputs, trace=False)[0]


if __name__ == "__main__":
    rng = np.random.default_rng(0)
    ins = {
        "white_features": rng.random((B, IN), dtype=np.float32),
        "black_features": rng.random((B, IN), dtype=np.float32),
        "side_to_move": np.ones((B,), dtype=np.float32),
        "W_ft": (0.1 * rng.standard_normal((H, IN))).astype(np.float32),
        "W1": (0.06 * rng.standard_normal((32, 2 * H))).astype(np.float32),
        "b1": np.zeros(32, np.float32),
        "W2": (0.17 * rng.standard_normal((32, 32))).astype(np.float32),
        "b2": np.zeros(32, np.float32),
        "W3": (0.24 * rng.standard_normal((1, 32))).astype(np.float32),
        "b3": np.zeros(1, np.float32),
    }
    out = kernel(**ins)
    # host reference
    whr = np.clip(ins["white_features"] @ ins["W_ft"].T, 0, 1)
    bhr = np.clip(ins["black_features"] @ ins["W_ft"].T, 0, 1)
    x = np.concatenate([whr, bhr], axis=1)
    x = np.maximum(x @ ins["W1"].T + ins["b1"], 0)
    x = np.maximum(x @ ins["W2"].T + ins["b2"], 0)
    ref = (x @ ins["W3"].T + ins["b3"]) * ins["side_to_move"][:, None]
    rel = np.linalg.norm(out - ref) / np.linalg.norm(ref)
    print("rel err:", rel)
